# revision 1
# baseline (speedup 1.0000x reference)
"""ColorINN forward kernel for 8 Trainium2 NeuronCores (pure data parallel).

Strategy:
- Batch B=524288 split evenly over 8 cores (Nc=65536 each), SPMD.
- Per core, the 4-feature coupling state stays SBUF-resident all 8 blocks
  as 32 per-tile [128, 512] fp16 tiles in a "span layout": partition
  32*j + r holds feature r of chunk j (chunk = 512 samples), so all small
  elementwise coupling work runs as full-width [128, 512] tiles and the
  only DRAM traffic is the initial load and final store.
- Each of the 8 coupling blocks runs as two passes over all tiles so the ACT
  table set only swaps twice per block (gelu+tanh set, then exp set):
    pass 1: L1 (K=2, row-packed via tile_position) -> gelu -> W2 (128x128)
            -> gelu -> W3a/W3b (M=4, col-strip packed) -> tanh -> stash
    pass 2: exp -> coupling mul/add -> 4x4 permute matmul (diagonal packed)
            -> +c bias -> store next state
- Matmuls run in fp16 (weights pre-cast on chip, activations produced as
  fp16 by ACT/DVE). Measured on hardware: rel err 1.3e-3, absmax 1.4e-2 on
  an output scale of ~7.8. A post-trace BIR pass legalizes sync waits for
  walrus codegen's one-wait-per-instruction caps (PE-self waits on matmuls
  are dropped as redundant; other overflow waits move to injected
  single-wait EventSemaphore instructions on the same engine).
"""

import os
import numpy as np

L = 8
H = 128
B = 524288
NCORES = 8
NC = B // NCORES          # samples per core
CHUNK = 512               # samples per chunk (one matmul stream / psum bank)
NCHUNK = 4                # chunks packed across partition strips
TILE = CHUNK * NCHUNK     # 2048 samples per tile
NT = NC // TILE           # 32 tiles per pass
HALF = NT // 2            # tiles per half-pass (bounds SBUF batch size)
SPAN = NC // NCHUNK       # 16384 span columns of DRAM state

# weight-stack column offsets
OW1 = 0
OW2 = OW1 + L * H
OW3A = OW2 + L * H
OW3B = OW3A + L * 4
OM = OW3B + L * 4
OB1 = OM + L * 4
OB2 = OB1 + L
OBT = OB2 + L
OCF = OBT + L
WCOLS = OCF + L

_ROWS_JR = (32 * np.arange(NCHUNK)[:, None] + np.arange(4)[None, :]).reshape(-1)


def _softplus(x, beta=1.0):
    x = np.asarray(x, np.float64)
    return np.log1p(np.exp(-np.abs(beta * x))) / beta + np.maximum(x, 0.0)


def _pack_weights(W1, b1, W2, b2, W3, b3, g, off, P):
    """Host-side constant folding -> one [128, WCOLS] f32 stack."""
    w = np.zeros((128, WCOLS), np.float32)
    for l in range(L):
        scale = 0.2 * _softplus(0.5 * g[l].astype(np.float64))          # (4,)
        M_mat = scale[:, None] * P[l].astype(np.float64).T              # [i,m] = scale_i * P[m,i]
        c = off[l].astype(np.float64) @ P[l].astype(np.float64).T
        b3s = 0.1 * b3[l].astype(np.float64)
        c_fold = c + np.array([0, 0, b3s[2], b3s[3]]) @ M_mat
        for j in range(NCHUNK):
            r0 = 32 * j
            # L1 lhsT rows {32j, 32j+1}: lhsT[r, m] = W1[m, r]
            w[r0:r0 + 2, OW1 + l * H:OW1 + (l + 1) * H] = W1[l].T
            # P-matmul lhsT rows {32j..32j+3}: lhsT[i, m] = M_mat[i, m]
            w[r0:r0 + 4, OM + l * 4:OM + (l + 1) * 4] = M_mat.astype(np.float32)
            # tanh bias rows {32j+2, 32j+3} = 0.1*b3[0:2]; elsewhere 0 so the
            # x1 rows see tanh(0)=0 -> exp=1 (x1 passthrough trick)
            w[r0 + 2:r0 + 4, OBT + l] = (0.1 * b3[l][0:2]).astype(np.float32)
            w[r0:r0 + 4, OCF + l] = c_fold.astype(np.float32)
        # W2 lhsT (all 128 rows): lhsT[k, m] = W2[m, k]
        w[:, OW2 + l * H:OW2 + (l + 1) * H] = W2[l].T
        # W3a/W3b lhsT [128, 4]: cols 0,1 zero; col 2+r = W3-row (a outputs land
        # on rows {32j+2, 32j+3}, aligned with x2 in the state span)
        w[:, OW3A + l * 4 + 2] = W3[l][0]
        w[:, OW3A + l * 4 + 3] = W3[l][1]
        w[:, OW3B + l * 4 + 2] = 0.1 * W3[l][2]
        w[:, OW3B + l * 4 + 3] = 0.1 * W3[l][3]
        w[:, OB1 + l] = b1[l]
        w[:, OB2 + l] = b2[l]
    return w


def _to_span(x4):
    """[4, NC] feature-major -> [128, SPAN] span layout."""
    s = np.zeros((128, SPAN), np.float32)
    x = x4.reshape(4, NT, NCHUNK, CHUNK)          # [r, g, j, c]
    s[_ROWS_JR, :] = x.transpose(2, 0, 1, 3).reshape(16, SPAN)  # [j, r, g, c]
    return s


def _from_span(s):
    """[128, SPAN] span layout -> [NC, 4] sample-major."""
    zs = s[_ROWS_JR, :].reshape(NCHUNK, 4, NT, CHUNK)   # [j, r, g, c]
    return zs.transpose(2, 0, 3, 1).reshape(NC, 4)


_PROGRAM = None


def _strip_pe_self_waits(bj_bytes):
    """Legalize sync waits for walrus codegen wait-slot caps.

    Most TRN2 instruction structs accept only one attached sync wait
    (Activation takes two). Tile can emit more. Two fixes, applied in order:
    - Matmults drop PE-self waits (PSUM WAW between matmuls is already
      guaranteed by in-order matmul completion on TRN2).
    - Any remaining overflow waits move onto an injected same-engine
      EventSemaphore placed immediately before the instruction.
    """
    import json
    bj = json.loads(bj_bytes)
    caps = {"EventSemaphore": 99, "Call": 99}
    nes = 0
    for f in bj["functions"]:
        for blk in f["blocks"]:
            out_insts = []
            for ins in blk["instructions"]:
                si = ins.get("sync_info") or {}
                w = si.get("on_wait") or []
                op = ins.get("opcode")
                if op == "Matmult" and len(w) >= 2:
                    w = [x for x in w
                         if not x.get("ant_name", "").startswith("PE")]
                    si["on_wait"] = w
                cap = caps.get(op, 1)
                if len(w) > cap:
                    keep = w[-cap:] if cap else []
                    moved = w[:-cap] if cap else list(w)
                    si["on_wait"] = keep
                    for mv in moved:
                        nes += 1
                        out_insts.append({
                            "debug": ins.get("debug", 0),
                            "engine": ins.get("engine"),
                            "ins": [], "outs": [],
                            "name": f"eswait_{nes}",
                            "opcode": "EventSemaphore",
                            "sync_info": {"on_update": [], "on_wait": [mv]},
                        })
                out_insts.append(ins)
            blk["instructions"] = out_insts
    return json.dumps(bj).encode(), nes


def _build_program():
    import concourse.bass as bass
    import concourse.tile as tile
    import concourse.mybir as mybir
    from contextlib import ExitStack

    f32 = mybir.dt.float32
    f32r = mybir.dt.float32r
    f16 = mybir.dt.float16
    AF = mybir.ActivationFunctionType

    nc = bass.Bass("TRN2", target_bir_lowering=False, debug=False)
    x0 = nc.dram_tensor("x0", [128, SPAN], f32, kind="ExternalInput").ap()
    wstk = nc.dram_tensor("wstk", [128, WCOLS], f32, kind="ExternalInput").ap()
    z = nc.dram_tensor("z", [128, SPAN], f32, kind="ExternalOutput").ap()

    def r32(ap):
        return ap.bitcast(f32r)

    with tile.TileContext(nc) as tc, ExitStack() as ctx:
        consts = ctx.enter_context(tc.tile_pool(name="consts", bufs=1))
        scr = ctx.enter_context(tc.tile_pool(name="scr", bufs=3))
        vtp = ctx.enter_context(tc.tile_pool(name="vt", bufs=1))
        hp = ctx.enter_context(tc.tile_pool(name="hp", bufs=2))
        batp = ctx.enter_context(tc.tile_pool(name="bat", bufs=1))
        pre_pool = ctx.enter_context(tc.tile_pool(name="pre", bufs=2, space="PSUM"))
        sm_pool = ctx.enter_context(tc.tile_pool(name="sm", bufs=1, space="PSUM"))
        out_pool = ctx.enter_context(tc.tile_pool(name="po", bufs=2, space="PSUM"))

        wsb = consts.tile([128, WCOLS], f32)
        nc.sync.dma_start(out=wsb[:, :], in_=wstk[:, :])
        wsb16 = consts.tile([128, WCOLS], f16)
        nc.vector.tensor_copy(wsb16[:, :], wsb[:, :])
        # tiny PE op consuming wsb so the weight-DMA wait lands here once,
        # not on the first real (fused-ldweights) matmul of every engine epoch
        warm = pre_pool.tile([128, 1024], f32, tag="pre")
        nc.tensor.matmul(warm[0:2, 0:2], wsb16[0:2, 0:2], wsb16[0:2, 0:2],
                         start=True, stop=True)
        warmsb = consts.tile([128, 2], f32)
        nc.scalar.copy(warmsb[0:1, 0:1], wsb[0:1, 0:1])
        nc.vector.tensor_copy(warmsb[0:1, 1:2], wsb[0:1, 1:2])

        vtiles = []
        for t in range(NT):
            vt = vtp.tile([128, CHUNK], f16, tag=f"v{t}")
            nc.gpsimd.dma_start(out=vt[:, :],
                                in_=x0[:, t * CHUNK:(t + 1) * CHUNK])
            vtiles.append(vt)

        for l in range(L):
            w1 = wsb16[:, OW1 + l * H:OW1 + (l + 1) * H]
            w2 = wsb16[:, OW2 + l * H:OW2 + (l + 1) * H]
            w3a = wsb16[:, OW3A + l * 4:OW3A + (l + 1) * 4]
            w3b = wsb16[:, OW3B + l * 4:OW3B + (l + 1) * 4]
            mw = wsb16[:, OM + l * 4:OM + (l + 1) * 4]
            b1ap = wsb[:, OB1 + l:OB1 + l + 1]
            b2ap = wsb[:, OB2 + l:OB2 + l + 1]
            btap = wsb[:, OBT + l:OBT + l + 1]
            cfap = wsb[:, OCF + l:OCF + l + 1]

            for half in range(2):
                tB = batp.tile([128, HALF * CHUNK], f32, tag="tB")
                a2B = batp.tile([128, HALF * CHUNK], f16, tag="a2B")
                tiles = range(half * HALF, (half + 1) * HALF)
                # ---- pass 1: gelu/tanh table set ----
                for t in tiles:
                    toff = (t - half * HALF) * CHUNK
                    xsp = vtiles[t]
                    h1 = hp.tile([128, TILE], f16, tag="h1")
                    for hh in range(2):
                        pre = pre_pool.tile([128, 1024], f32, tag="pre")
                        for jj in range(2):
                            j = hh * 2 + jj
                            nc.tensor.matmul(
                                pre[:, jj * 512:(jj + 1) * 512],
                                w1[32 * j:32 * j + 2, :],
                                xsp[32 * j:32 * j + 2, :],
                                start=True, stop=True,
                                tile_position=(32 * j, 0))
                        nc.scalar.activation(
                            h1[:, hh * 1024:(hh + 1) * 1024], pre[:, :],
                            AF.Gelu, bias=b1ap, scale=1.0)
                    h2 = hp.tile([128, TILE], f16, tag="h2")
                    for hh in range(2):
                        pre = pre_pool.tile([128, 1024], f32, tag="pre")
                        for jj in range(2):
                            j = hh * 2 + jj
                            nc.tensor.matmul(
                                pre[:, jj * 512:(jj + 1) * 512],
                                w2,
                                h1[:, j * 512:(j + 1) * 512],
                                start=True, stop=True)
                        nc.scalar.activation(
                            h2[:, hh * 1024:(hh + 1) * 1024], pre[:, :],
                            AF.Gelu, bias=b2ap, scale=1.0)
                    a1ps = sm_pool.tile([128, CHUNK], f32, tag="a1")
                    a2ps = sm_pool.tile([128, CHUNK], f32, tag="a2")
                    for j in range(4):
                        nc.tensor.matmul(
                            a1ps[32 * j:32 * j + 4, :], w3a,
                            h2[:, j * 512:(j + 1) * 512],
                            start=True, stop=True, tile_position=(0, 32 * j))
                    for j in range(4):
                        nc.tensor.matmul(
                            a2ps[32 * j:32 * j + 4, :], w3b,
                            h2[:, j * 512:(j + 1) * 512],
                            start=True, stop=True, tile_position=(0, 32 * j))
                    nc.scalar.activation(tB[:, toff:toff + CHUNK], a1ps[:, :],
                                         AF.Tanh, bias=btap, scale=0.1)
                    nc.vector.tensor_copy(a2B[:, toff:toff + CHUNK], a2ps[:, :])
                # ---- pass 2: exp table set ----
                for t in tiles:
                    toff = (t - half * HALF) * CHUNK
                    vt = vtiles[t]
                    esp = scr.tile([128, CHUNK], f16, tag="esp")
                    nc.scalar.activation(esp[:, :], tB[:, toff:toff + CHUNK],
                                         AF.Exp, scale=2.0)
                    xe = scr.tile([128, CHUNK], f16, tag="xe")
                    nc.vector.tensor_mul(xe[:, :], vt[:, :], esp[:, :])
                    # x1 rows: e==1 and a2==0, so this leaves x1 intact
                    nc.vector.tensor_add(vt[:, :], xe[:, :],
                                         a2B[:, toff:toff + CHUNK])
                    vops = out_pool.tile([128, CHUNK], f32, tag="vo")
                    for j in range(4):
                        nc.tensor.matmul(
                            vops[32 * j:32 * j + 4, :],
                            mw[32 * j:32 * j + 4, :],
                            vt[32 * j:32 * j + 4, :],
                            start=True, stop=True,
                            tile_position=(32 * j, 32 * j))
                    nc.vector.tensor_scalar_add(vt[:, :], vops[:, :], cfap)
                    if l == L - 1:
                        nc.gpsimd.dma_start(out=z[:, t * CHUNK:(t + 1) * CHUNK],
                                            in_=vt[:, :])
    return nc


def _get_program():
    global _PROGRAM
    if _PROGRAM is None:
        nc = _build_program()
        fixed, _ = _strip_pe_self_waits(nc.to_json_bytes())
        nc.to_json_bytes = lambda: fixed
        _PROGRAM = nc
    return _PROGRAM


LAST_EXEC_NS = None


def kernel(XYZ, W1, b1, W2, b2, W3, b3, g, off, P):
    global LAST_EXEC_NS
    from concourse import bass_utils

    XYZ = np.ascontiguousarray(XYZ, np.float32)
    wstk = _pack_weights(np.asarray(W1), np.asarray(b1), np.asarray(W2),
                         np.asarray(b2), np.asarray(W3), np.asarray(b3),
                         np.asarray(g), np.asarray(off), np.asarray(P))
    in_maps = []
    for c in range(NCORES):
        x4 = np.zeros((4, NC), np.float32)
        x4[:3] = XYZ[c * NC:(c + 1) * NC].T
        in_maps.append({"x0": _to_span(x4), "wstk": wstk})

    nc = _get_program()
    trace = bool(int(os.environ.get("COLORINN_TRACE", "0")))
    res = bass_utils.run_bass_kernel_spmd(
        nc, in_maps, core_ids=list(range(NCORES)), trace=trace)
    LAST_EXEC_NS = res.exec_time_ns

    out = np.empty((B, 3), np.float32)
    for c in range(NCORES):
        out[c * NC:(c + 1) * NC] = _from_span(res.results[c]["z"])[:, :3]
    return out



# revision 2
# speedup vs baseline: 13.3239x; 13.3239x over previous
"""ColorINN forward kernel for 8 Trainium2 NeuronCores (pure data parallel).

Strategy:
- Batch B=524288 split evenly over 8 cores (Nc=65536 each), SPMD.
- Per core, the 4-feature coupling state stays SBUF-resident all 8 blocks
  as 32 per-tile [128, 512] fp16 tiles in a "span layout": partition
  32*j + r holds feature r of chunk j (chunk = 512 samples), so all small
  elementwise coupling work runs as full-width [128, 512] tiles and the
  only DRAM traffic is the initial load and final store.
- Each of the 8 coupling blocks runs as two passes over all tiles so the ACT
  table set only swaps twice per block (gelu+tanh set, then exp set):
    pass 1: L1 (K=2, row-packed via tile_position) -> gelu -> W2 (128x128)
            -> gelu -> W3a/W3b (M=4, col-strip packed) -> tanh -> stash
    pass 2: exp -> coupling mul/add -> 4x4 permute matmul (diagonal packed)
            -> +c bias -> store next state
- Host <-> device traffic is minimized (it rides a slow tunnel): inputs are
  shipped as XYZ^T in fp16 [3, Nc] plus a compact fp16/fp32 weight stack;
  the span layout and the strip-replicated weight tiles are built on-device
  with small DMAs. Output returns as [3, Nc] fp16.
- The JAX persistent compilation cache is enabled so repeat calls skip the
  per-call XLA executable rebuild (the jit closure is fresh each call).
- Matmuls run in fp16. Measured on hardware: rel err ~1.3e-3 on an output
  scale of ~7.8. A post-trace BIR pass legalizes sync waits for walrus
  codegen's one-wait-per-instruction caps (PE-self waits on matmuls are
  dropped as redundant; other overflow waits move to injected single-wait
  EventSemaphore instructions on the same engine).
"""

import os
import numpy as np

L = 8
H = 128
B = 524288
NCORES = 8
NC = B // NCORES          # samples per core
CHUNK = 512               # samples per chunk (one matmul stream / psum bank)
NCHUNK = 4                # chunks packed across partition strips
TILE = CHUNK * NCHUNK     # 2048 samples per tile
NT = NC // TILE           # 32 tiles per pass
HALF = NT // 2            # tiles per half-pass (bounds SBUF batch size)

# w2s (fp16) column layout: [0, 1024) W2 lhsT per block; [1024, 1152)
# rows 2l+r carry W1[l].T (compact, expanded on-device)
W2COLS = L * H + H
# wrow (fp32) column layout
OB1 = 0            # 8 cols: b1 per block (dense 128 rows)
OB2 = 8            # 8 cols: b2
OW3 = 16           # 32 cols: l*4 + {W3[l][0], W3[l][1], .1*W3[l][2], .1*W3[l][3]}
OMW = 48           # 32 cols, rows 0-3: M_mat per block (strip-expanded on-device)
OBT = 80           # 8 cols, rows 0-1: 0.1*b3[l][0:2] (-> strip rows +2,+3)
OCF = 88           # 8 cols, rows 0-3: folded output bias
WRCOLS = 96


def _softplus(x, beta=1.0):
    x = np.asarray(x, np.float64)
    return np.log1p(np.exp(-np.abs(beta * x))) / beta + np.maximum(x, 0.0)


def _pack_weights(W1, b1, W2, b2, W3, b3, g, off, P):
    """Host-side constant folding -> compact fp16 + fp32 stacks."""
    w2s = np.zeros((128, W2COLS), np.float16)
    wrow = np.zeros((128, WRCOLS), np.float32)
    for l in range(L):
        scale = 0.2 * _softplus(0.5 * g[l].astype(np.float64))          # (4,)
        M_mat = scale[:, None] * P[l].astype(np.float64).T              # [i,m] = scale_i * P[m,i]
        c = off[l].astype(np.float64) @ P[l].astype(np.float64).T
        b3s = 0.1 * b3[l].astype(np.float64)
        c_fold = c + np.array([0, 0, b3s[2], b3s[3]]) @ M_mat
        w2s[:, l * H:(l + 1) * H] = W2[l].T
        w2s[2 * l:2 * l + 2, L * H:] = W1[l].T
        wrow[:, OB1 + l] = b1[l]
        wrow[:, OB2 + l] = b2[l]
        wrow[:, OW3 + l * 4 + 0] = W3[l][0]
        wrow[:, OW3 + l * 4 + 1] = W3[l][1]
        wrow[:, OW3 + l * 4 + 2] = 0.1 * W3[l][2]
        wrow[:, OW3 + l * 4 + 3] = 0.1 * W3[l][3]
        wrow[0:4, OMW + l * 4:OMW + (l + 1) * 4] = M_mat.astype(np.float32)
        wrow[0:2, OBT + l] = (0.1 * b3[l][0:2]).astype(np.float32)
        wrow[0:4, OCF + l] = c_fold.astype(np.float32)
    return w2s, wrow


_PROGRAM = None
_JAX_CACHE_SET = False


def _set_jax_cache():
    """Persistent XLA compilation cache: repeat kernel() calls rebuild the
    jit closure inside run_bass_kernel_spmd, so without this every call
    pays ~0.7s of executable rebuild."""
    global _JAX_CACHE_SET
    if _JAX_CACHE_SET:
        return
    try:
        import jax
        jax.config.update("jax_compilation_cache_dir", "/tmp/colorinn_jaxcache")
        jax.config.update("jax_persistent_cache_min_compile_time_secs", 0.0)
        jax.config.update("jax_persistent_cache_min_entry_size_bytes", -1)
    except Exception:
        pass
    _JAX_CACHE_SET = True


def _strip_pe_self_waits(bj_bytes):
    """Legalize sync waits for walrus codegen wait-slot caps.

    Most TRN2 instruction structs accept only one attached sync wait
    (Activation takes two). Tile can emit more. Two fixes, applied in order:
    - Matmults drop PE-self waits (PSUM WAW between matmuls is already
      guaranteed by in-order matmul completion on TRN2).
    - Any remaining overflow waits move onto an injected same-engine
      EventSemaphore placed immediately before the instruction.
    """
    import json
    bj = json.loads(bj_bytes)
    caps = {"EventSemaphore": 99, "Call": 99}
    nes = 0
    for f in bj["functions"]:
        for blk in f["blocks"]:
            out_insts = []
            for ins in blk["instructions"]:
                si = ins.get("sync_info") or {}
                w = si.get("on_wait") or []
                op = ins.get("opcode")
                if op == "Matmult" and len(w) >= 2:
                    w = [x for x in w
                         if not x.get("ant_name", "").startswith("PE")]
                    si["on_wait"] = w
                cap = caps.get(op, 1)
                if len(w) > cap:
                    keep = w[-cap:] if cap else []
                    moved = w[:-cap] if cap else list(w)
                    si["on_wait"] = keep
                    for mv in moved:
                        nes += 1
                        out_insts.append({
                            "debug": ins.get("debug", 0),
                            "engine": ins.get("engine"),
                            "ins": [], "outs": [],
                            "name": f"eswait_{nes}",
                            "opcode": "EventSemaphore",
                            "sync_info": {"on_update": [], "on_wait": [mv]},
                        })
                out_insts.append(ins)
            blk["instructions"] = out_insts
    return json.dumps(bj).encode(), nes


def _build_program():
    import concourse.bass as bass
    import concourse.tile as tile
    import concourse.mybir as mybir
    from contextlib import ExitStack

    f32 = mybir.dt.float32
    f16 = mybir.dt.float16
    AF = mybir.ActivationFunctionType

    nc = bass.Bass("TRN2", target_bir_lowering=False, debug=False)
    xt = nc.dram_tensor("xt", [3, NC], f16, kind="ExternalInput").ap()
    w2d = nc.dram_tensor("w2s", [128, W2COLS], f16, kind="ExternalInput").ap()
    wrd = nc.dram_tensor("wrow", [128, WRCOLS], f32, kind="ExternalInput").ap()
    z3 = nc.dram_tensor("z3", [3, NC], f16, kind="ExternalOutput").ap()

    with tile.TileContext(nc) as tc, ExitStack() as ctx:
        consts = ctx.enter_context(tc.tile_pool(name="consts", bufs=1))
        scr = ctx.enter_context(tc.tile_pool(name="scr", bufs=3))
        vtp = ctx.enter_context(tc.tile_pool(name="vt", bufs=1))
        hp = ctx.enter_context(tc.tile_pool(name="hp", bufs=2))
        batp = ctx.enter_context(tc.tile_pool(name="bat", bufs=1))
        pre_pool = ctx.enter_context(tc.tile_pool(name="pre", bufs=2, space="PSUM"))
        sm_pool = ctx.enter_context(tc.tile_pool(name="sm", bufs=1, space="PSUM"))
        out_pool = ctx.enter_context(tc.tile_pool(name="po", bufs=2, space="PSUM"))

        # ---- weight load + on-device expansion ----
        w2sb = consts.tile([128, W2COLS], f16)
        nc.sync.dma_start(out=w2sb[:, :], in_=w2d[:, :])
        wrsb = consts.tile([128, WRCOLS], f32)
        nc.sync.dma_start(out=wrsb[:, :], in_=wrd[:, :])

        # tiny ops consuming the weight DMAs so their waits land here once,
        # not on the first real instruction of every engine epoch
        warm = pre_pool.tile([128, 1024], f32, tag="pre")
        nc.tensor.matmul(warm[0:2, 0:2], w2sb[0:2, 0:2], w2sb[0:2, 0:2],
                         start=True, stop=True)
        warmsb = consts.tile([128, 2], f32)
        nc.scalar.copy(warmsb[0:1, 0:1], wrsb[0:1, 0:1])
        nc.vector.tensor_copy(warmsb[0:1, 1:2], wrsb[0:1, 1:2])

        # W1 lhsT rows {32j, 32j+1} per block, from compact rows 2l+r
        w116 = consts.tile([128, L * H], f16)
        for l in range(L):
            for j in range(NCHUNK):
                nc.scalar.dma_start(
                    out=w116[32 * j:32 * j + 2, l * H:(l + 1) * H],
                    in_=w2sb[2 * l:2 * l + 2, L * H:])
        # W3a/W3b lhsT [128, 4] per block: cols 0,1 zero; col 2+r = W3-row
        # (a outputs land on rows {32j+2, 32j+3}, aligned with x2 in the span)
        w3ab = consts.tile([128, 64], f16)
        nc.vector.memset(w3ab[:, :], 0.0)
        for l in range(L):
            nc.vector.tensor_copy(w3ab[:, l * 4 + 2:l * 4 + 4],
                                  wrsb[:, OW3 + l * 4:OW3 + l * 4 + 2])
            nc.vector.tensor_copy(w3ab[:, 32 + l * 4 + 2:32 + l * 4 + 4],
                                  wrsb[:, OW3 + l * 4 + 2:OW3 + l * 4 + 4])
        # P-matmul lhsT rows {32j..32j+3}: M_mat, strip-replicated
        mw16 = consts.tile([128, 32], f16)
        nc.vector.tensor_copy(mw16[0:4, :], wrsb[0:4, OMW:OMW + 32])
        for j in range(1, NCHUNK):
            nc.sync.dma_start(out=mw16[32 * j:32 * j + 4, :], in_=mw16[0:4, :])
        # tanh bias rows {32j+2, 32j+3} = 0.1*b3[0:2]; elsewhere 0 so the
        # x1 rows see tanh(0)=0 -> exp=1 (x1 passthrough trick)
        btf = consts.tile([128, L], f32)
        nc.vector.memset(btf[:, :], 0.0)
        cff = consts.tile([128, L], f32)
        nc.vector.memset(cff[:, :], 0.0)
        for j in range(NCHUNK):
            nc.sync.dma_start(out=btf[32 * j + 2:32 * j + 4, :],
                              in_=wrsb[0:2, OBT:OBT + L])
            nc.sync.dma_start(out=cff[32 * j:32 * j + 4, :],
                              in_=wrsb[0:4, OCF:OCF + L])

        # ---- input load: span layout built by DMA from XYZ^T fp16 ----
        vtiles = []
        for t in range(NT):
            vt = vtp.tile([128, CHUNK], f16, tag=f"v{t}")
            nc.vector.memset(vt[:, :], 0.0)   # pad feature row starts at 0
            for j in range(NCHUNK):
                c0 = t * TILE + j * CHUNK
                nc.sync.dma_start(out=vt[32 * j:32 * j + 3, :],
                                  in_=xt[:, c0:c0 + CHUNK])
            vtiles.append(vt)

        for l in range(L):
            w1 = w116[:, l * H:(l + 1) * H]
            w2 = w2sb[:, l * H:(l + 1) * H]
            w3a = w3ab[:, l * 4:(l + 1) * 4]
            w3b = w3ab[:, 32 + l * 4:32 + (l + 1) * 4]
            mw = mw16[:, l * 4:(l + 1) * 4]
            b1ap = wrsb[:, OB1 + l:OB1 + l + 1]
            b2ap = wrsb[:, OB2 + l:OB2 + l + 1]
            btap = btf[:, l:l + 1]
            cfap = cff[:, l:l + 1]

            for half in range(2):
                tB = batp.tile([128, HALF * CHUNK], f32, tag="tB")
                a2B = batp.tile([128, HALF * CHUNK], f16, tag="a2B")
                tiles = range(half * HALF, (half + 1) * HALF)
                # ---- pass 1: gelu/tanh table set ----
                for t in tiles:
                    toff = (t - half * HALF) * CHUNK
                    xsp = vtiles[t]
                    h1 = hp.tile([128, TILE], f16, tag="h1")
                    for hh in range(2):
                        pre = pre_pool.tile([128, 1024], f32, tag="pre")
                        for jj in range(2):
                            j = hh * 2 + jj
                            nc.tensor.matmul(
                                pre[:, jj * 512:(jj + 1) * 512],
                                w1[32 * j:32 * j + 2, :],
                                xsp[32 * j:32 * j + 2, :],
                                start=True, stop=True,
                                tile_position=(32 * j, 0))
                        nc.scalar.activation(
                            h1[:, hh * 1024:(hh + 1) * 1024], pre[:, :],
                            AF.Gelu, bias=b1ap, scale=1.0)
                    h2 = hp.tile([128, TILE], f16, tag="h2")
                    for hh in range(2):
                        pre = pre_pool.tile([128, 1024], f32, tag="pre")
                        for jj in range(2):
                            j = hh * 2 + jj
                            nc.tensor.matmul(
                                pre[:, jj * 512:(jj + 1) * 512],
                                w2,
                                h1[:, j * 512:(j + 1) * 512],
                                start=True, stop=True)
                        nc.scalar.activation(
                            h2[:, hh * 1024:(hh + 1) * 1024], pre[:, :],
                            AF.Gelu, bias=b2ap, scale=1.0)
                    a1ps = sm_pool.tile([128, CHUNK], f32, tag="a1")
                    a2ps = sm_pool.tile([128, CHUNK], f32, tag="a2")
                    for j in range(4):
                        nc.tensor.matmul(
                            a1ps[32 * j:32 * j + 4, :], w3a,
                            h2[:, j * 512:(j + 1) * 512],
                            start=True, stop=True, tile_position=(0, 32 * j))
                    for j in range(4):
                        nc.tensor.matmul(
                            a2ps[32 * j:32 * j + 4, :], w3b,
                            h2[:, j * 512:(j + 1) * 512],
                            start=True, stop=True, tile_position=(0, 32 * j))
                    nc.scalar.activation(tB[:, toff:toff + CHUNK], a1ps[:, :],
                                         AF.Tanh, bias=btap, scale=0.1)
                    nc.vector.tensor_copy(a2B[:, toff:toff + CHUNK], a2ps[:, :])
                # ---- pass 2: exp table set ----
                for t in tiles:
                    toff = (t - half * HALF) * CHUNK
                    vt = vtiles[t]
                    esp = scr.tile([128, CHUNK], f16, tag="esp")
                    nc.scalar.activation(esp[:, :], tB[:, toff:toff + CHUNK],
                                         AF.Exp, scale=2.0)
                    xe = scr.tile([128, CHUNK], f16, tag="xe")
                    nc.vector.tensor_mul(xe[:, :], vt[:, :], esp[:, :])
                    # x1 rows: e==1 and a2==0, so this leaves x1 intact
                    nc.vector.tensor_add(vt[:, :], xe[:, :],
                                         a2B[:, toff:toff + CHUNK])
                    vops = out_pool.tile([128, CHUNK], f32, tag="vo")
                    for j in range(4):
                        nc.tensor.matmul(
                            vops[32 * j:32 * j + 4, :],
                            mw[32 * j:32 * j + 4, :],
                            vt[32 * j:32 * j + 4, :],
                            start=True, stop=True,
                            tile_position=(32 * j, 32 * j))
                    nc.vector.tensor_scalar_add(vt[:, :], vops[:, :], cfap)
                    if l == L - 1:
                        for j in range(NCHUNK):
                            c0 = t * TILE + j * CHUNK
                            nc.sync.dma_start(out=z3[:, c0:c0 + CHUNK],
                                              in_=vt[32 * j:32 * j + 3, :])
    return nc


def _get_program():
    global _PROGRAM
    if _PROGRAM is None:
        nc = _build_program()
        fixed, _ = _strip_pe_self_waits(nc.to_json_bytes())
        nc.to_json_bytes = lambda: fixed
        _PROGRAM = nc
    return _PROGRAM


LAST_EXEC_NS = None


def kernel(XYZ, W1, b1, W2, b2, W3, b3, g, off, P):
    global LAST_EXEC_NS
    _set_jax_cache()
    from concourse import bass_utils

    XYZ = np.asarray(XYZ, np.float32)
    w2s, wrow = _pack_weights(np.asarray(W1), np.asarray(b1), np.asarray(W2),
                              np.asarray(b2), np.asarray(W3), np.asarray(b3),
                              np.asarray(g), np.asarray(off), np.asarray(P))
    XT = XYZ.T.astype(np.float16)        # [3, B], contiguous
    in_maps = [{"xt": XT[:, c * NC:(c + 1) * NC], "w2s": w2s, "wrow": wrow}
               for c in range(NCORES)]

    nc = _get_program()
    trace = bool(int(os.environ.get("COLORINN_TRACE", "0")))
    res = bass_utils.run_bass_kernel_spmd(
        nc, in_maps, core_ids=list(range(NCORES)), trace=trace)
    LAST_EXEC_NS = res.exec_time_ns

    out = np.empty((B, 3), np.float32)
    for c in range(NCORES):
        out[c * NC:(c + 1) * NC] = res.results[c]["z3"].T
    return out


# revision 8
# speedup vs baseline: 14.4994x; 1.0882x over previous
"""ColorINN forward kernel for 8 Trainium2 NeuronCores (pure data parallel).

Strategy:
- Batch B=524288 split evenly over 8 cores (Nc=65536 each), SPMD.
- Per core, the 4-feature coupling state stays SBUF-resident all 8 blocks
  as 32 per-tile [128, 512] fp16 tiles in a "span layout": partition
  32*j + r holds feature r of chunk j (chunk = 512 samples), so all small
  elementwise coupling work runs as full-width [128, 512] tiles and the
  only DRAM traffic is the initial load and final store.
- Each of the 8 coupling blocks runs as two passes over all tiles so the ACT
  table set only swaps twice per block (gelu+tanh set, then exp set):
    pass 1: L1 (K=2, row-packed via tile_position) -> gelu -> W2 (128x128)
            -> gelu -> W3a/W3b (M=4, col-strip packed) -> tanh -> stash
    pass 2: exp -> coupling mul/add -> 4x4 permute matmul (diagonal packed)
            -> +c bias -> store next state
- Host <-> device traffic is minimized (it rides a slow tunnel): inputs are
  shipped as XYZ^T in fp16 [3, Nc] plus a compact fp16/fp32 weight stack;
  the span layout and the strip-replicated weight tiles are built on-device
  with small DMAs. Output returns as [3, Nc] fp16.
- The JAX persistent compilation cache is enabled so repeat calls skip the
  per-call XLA executable rebuild (the jit closure is fresh each call).
- Matmuls run in fp16. Measured on hardware: rel err ~1.3e-3 on an output
  scale of ~7.8. A post-trace BIR pass legalizes sync waits for walrus
  codegen's one-wait-per-instruction caps (PE-self waits on matmuls are
  dropped as redundant; other overflow waits move to injected single-wait
  EventSemaphore instructions on the same engine).
"""

import os
import numpy as np

L = 8
H = 128
B = 524288
NCORES = 8
NC = B // NCORES          # samples per core
CHUNK = 512               # samples per chunk (one matmul stream / psum bank)
NCHUNK = 4                # chunks packed across partition strips
TILE = CHUNK * NCHUNK     # 2048 samples per tile
NT = NC // TILE           # 32 tiles per pass
HALF = NT // 2            # tiles per half-pass (bounds SBUF batch size)

# w28 (fp8 e4m3): [128, L*H] W2 lhsT per block (upcast to fp16 on-device)
# wrow (fp16) column layout
OB1 = 0            # 8 cols: b1 per block (dense 128 rows; cast f32 on-device)
OB2 = 8            # 8 cols: b2
OW3 = 16           # 32 cols: l*4 + {W3[l][0], W3[l][1], .1*W3[l][2], .1*W3[l][3]}
OMW = 48           # 32 cols, rows 0-3: M_mat per block (strip-expanded on-device)
OBT = 80           # 8 cols, rows 0-1: 0.1*b3[l][0:2] (-> strip rows +2,+3)
OCF = 88           # 8 cols, rows 0-3: folded output bias
OW1 = 96           # 128 cols, rows 0-15: row 2l+r = W1[l].T[r] (strip-expanded)
WRCOLS = 224


def _softplus(x, beta=1.0):
    x = np.asarray(x, np.float64)
    return np.log1p(np.exp(-np.abs(beta * x))) / beta + np.maximum(x, 0.0)


def _pack_weights(W1, b1, W2, b2, W3, b3, g, off, P):
    """Host-side constant folding -> compact fp8 + fp16 stacks."""
    import ml_dtypes
    w28 = np.zeros((128, L * H), ml_dtypes.float8_e4m3)
    wrow = np.zeros((128, WRCOLS), np.float16)
    for l in range(L):
        scale = 0.2 * _softplus(0.5 * g[l].astype(np.float64))          # (4,)
        M_mat = scale[:, None] * P[l].astype(np.float64).T              # [i,m] = scale_i * P[m,i]
        c = off[l].astype(np.float64) @ P[l].astype(np.float64).T
        b3s = 0.1 * b3[l].astype(np.float64)
        c_fold = c + np.array([0, 0, b3s[2], b3s[3]]) @ M_mat
        w28[:, l * H:(l + 1) * H] = W2[l].T.astype(ml_dtypes.float8_e4m3)
        wrow[2 * l:2 * l + 2, OW1:] = W1[l].T
        wrow[:, OB1 + l] = b1[l]
        wrow[:, OB2 + l] = b2[l]
        wrow[:, OW3 + l * 4 + 0] = W3[l][0]
        wrow[:, OW3 + l * 4 + 1] = W3[l][1]
        wrow[:, OW3 + l * 4 + 2] = 0.1 * W3[l][2]
        wrow[:, OW3 + l * 4 + 3] = 0.1 * W3[l][3]
        wrow[0:4, OMW + l * 4:OMW + (l + 1) * 4] = M_mat.astype(np.float16)
        wrow[0:2, OBT + l] = (0.1 * b3[l][0:2]).astype(np.float16)
        wrow[0:4, OCF + l] = c_fold.astype(np.float16)
    return w28, wrow


_PROGRAM = None
_JAX_CACHE_SET = False


def _set_jax_cache():
    """Persistent XLA compilation cache: repeat kernel() calls rebuild the
    jit closure inside run_bass_kernel_spmd, so without this every call
    pays ~0.7s of executable rebuild."""
    global _JAX_CACHE_SET
    if _JAX_CACHE_SET:
        return
    try:
        import jax
        jax.config.update("jax_compilation_cache_dir", "/tmp/colorinn_jaxcache")
        jax.config.update("jax_persistent_cache_min_compile_time_secs", 0.0)
        jax.config.update("jax_persistent_cache_min_entry_size_bytes", -1)
    except Exception:
        pass
    _JAX_CACHE_SET = True


def _strip_pe_self_waits(bj_bytes):
    """Legalize sync waits for walrus codegen wait-slot caps.

    Most TRN2 instruction structs accept only one attached sync wait
    (Activation takes two). Tile can emit more. Two fixes, applied in order:
    - Matmults drop PE-self waits (PSUM WAW between matmuls is already
      guaranteed by in-order matmul completion on TRN2).
    - Any remaining overflow waits move onto an injected same-engine
      EventSemaphore placed immediately before the instruction.
    """
    import json
    bj = json.loads(bj_bytes)
    caps = {"EventSemaphore": 99, "Call": 99}
    nes = 0
    for f in bj["functions"]:
        for blk in f["blocks"]:
            out_insts = []
            for ins in blk["instructions"]:
                si = ins.get("sync_info") or {}
                w = si.get("on_wait") or []
                op = ins.get("opcode")
                if op == "Matmult" and len(w) >= 2:
                    w = [x for x in w
                         if not x.get("ant_name", "").startswith("PE")]
                    si["on_wait"] = w
                cap = caps.get(op, 1)
                if len(w) > cap:
                    keep = w[-cap:] if cap else []
                    moved = w[:-cap] if cap else list(w)
                    si["on_wait"] = keep
                    for mv in moved:
                        nes += 1
                        out_insts.append({
                            "debug": ins.get("debug", 0),
                            "engine": ins.get("engine"),
                            "ins": [], "outs": [],
                            "name": f"eswait_{nes}",
                            "opcode": "EventSemaphore",
                            "sync_info": {"on_update": [], "on_wait": [mv]},
                        })
                out_insts.append(ins)
            blk["instructions"] = out_insts
    return json.dumps(bj).encode(), nes


def _build_program():
    import concourse.bass as bass
    import concourse.tile as tile
    import concourse.mybir as mybir
    from contextlib import ExitStack

    f32 = mybir.dt.float32
    f16 = mybir.dt.float16
    f8 = mybir.dt.float8e4
    AF = mybir.ActivationFunctionType

    nc = bass.Bass("TRN2", target_bir_lowering=False, debug=False)
    xt = nc.dram_tensor("xt", [3, NC], f16, kind="ExternalInput").ap()
    w2d = nc.dram_tensor("w28", [128, L * H], f8, kind="ExternalInput").ap()
    wrd = nc.dram_tensor("wrow", [128, WRCOLS], f16, kind="ExternalInput").ap()
    z3 = nc.dram_tensor("z3", [3, NC], f16, kind="ExternalOutput").ap()

    with tile.TileContext(nc) as tc, ExitStack() as ctx:
        consts = ctx.enter_context(tc.tile_pool(name="consts", bufs=1))
        scr = ctx.enter_context(tc.tile_pool(name="scr", bufs=3))
        vtp = ctx.enter_context(tc.tile_pool(name="vt", bufs=1))
        hp = ctx.enter_context(tc.tile_pool(name="hp", bufs=2))
        batp = ctx.enter_context(tc.tile_pool(name="bat", bufs=1))
        pre_pool = ctx.enter_context(tc.tile_pool(name="pre", bufs=2, space="PSUM"))
        sm_pool = ctx.enter_context(tc.tile_pool(name="sm", bufs=1, space="PSUM"))
        out_pool = ctx.enter_context(tc.tile_pool(name="po", bufs=2, space="PSUM"))

        # ---- weight load + on-device expansion ----
        w28sb = consts.tile([128, L * H], f8)
        nc.sync.dma_start(out=w28sb[:, :], in_=w2d[:, :])
        wrsb = consts.tile([128, WRCOLS], f16)
        nc.sync.dma_start(out=wrsb[:, :], in_=wrd[:, :])

        # upcast W2 fp8 -> fp16 for the matmuls
        w2sb = consts.tile([128, L * H], f16)
        nc.vector.tensor_copy(w2sb[:, :], w28sb[:, :])

        # tiny ops consuming the weight DMAs so their waits land here once,
        # not on the first real instruction of every engine epoch
        warm = pre_pool.tile([128, 1024], f32, tag="pre")
        nc.tensor.matmul(warm[0:2, 0:2], w2sb[0:2, 0:2], w2sb[0:2, 0:2],
                         start=True, stop=True)
        warmsb = consts.tile([128, 2], f32)
        nc.scalar.copy(warmsb[0:1, 0:1], wrsb[0:1, 0:1])

        # biases to f32 for the ACT bias APs
        bbf = consts.tile([128, 16], f32)
        nc.vector.tensor_copy(bbf[:, :], wrsb[:, OB1:OB1 + 16])
        # bt/cf compact rows cast to f32 (strip-expanded below)
        btcf = consts.tile([128, 16], f32)
        nc.vector.tensor_copy(btcf[0:2, 0:L], wrsb[0:2, OBT:OBT + L])
        nc.vector.tensor_copy(btcf[0:4, 8:8 + L], wrsb[0:4, OCF:OCF + L])

        # W1 lhsT rows {32j, 32j+1} per block, from compact rows 2l+r
        w116 = consts.tile([128, L * H], f16)
        for l in range(L):
            for j in range(NCHUNK):
                nc.scalar.dma_start(
                    out=w116[32 * j:32 * j + 2, l * H:(l + 1) * H],
                    in_=wrsb[2 * l:2 * l + 2, OW1:])
        # W3a/W3b lhsT [128, 4] per block: cols 0,1 zero; col 2+r = W3-row
        # (a outputs land on rows {32j+2, 32j+3}, aligned with x2 in the span)
        w3ab = consts.tile([128, 64], f16)
        nc.vector.memset(w3ab[:, :], 0.0)
        for l in range(L):
            nc.vector.tensor_copy(w3ab[:, l * 4 + 2:l * 4 + 4],
                                  wrsb[:, OW3 + l * 4:OW3 + l * 4 + 2])
            nc.vector.tensor_copy(w3ab[:, 32 + l * 4 + 2:32 + l * 4 + 4],
                                  wrsb[:, OW3 + l * 4 + 2:OW3 + l * 4 + 4])
        # P-matmul lhsT rows {32j..32j+3}: M_mat, strip-replicated
        mw16 = consts.tile([128, 32], f16)
        nc.vector.tensor_copy(mw16[0:4, :], wrsb[0:4, OMW:OMW + 32])
        for j in range(1, NCHUNK):
            nc.sync.dma_start(out=mw16[32 * j:32 * j + 4, :], in_=mw16[0:4, :])
        # tanh bias rows {32j+2, 32j+3} = 0.1*b3[0:2]; elsewhere 0 so the
        # x1 rows see tanh(0)=0 -> exp=1 (x1 passthrough trick)
        btf = consts.tile([128, L], f32)
        nc.vector.memset(btf[:, :], 0.0)
        cff = consts.tile([128, L], f32)
        nc.vector.memset(cff[:, :], 0.0)
        for j in range(NCHUNK):
            nc.sync.dma_start(out=btf[32 * j + 2:32 * j + 4, :],
                              in_=btcf[0:2, 0:L])
            nc.sync.dma_start(out=cff[32 * j:32 * j + 4, :],
                              in_=btcf[0:4, 8:8 + L])

        # ---- input load: span layout built by DMA from XYZ^T fp16 ----
        vtiles = []
        for t in range(NT):
            vt = vtp.tile([128, CHUNK], f16, tag=f"v{t}")
            nc.vector.memset(vt[:, :], 0.0)   # pad feature row starts at 0
            for j in range(NCHUNK):
                c0 = t * TILE + j * CHUNK
                nc.sync.dma_start(out=vt[32 * j:32 * j + 3, :],
                                  in_=xt[:, c0:c0 + CHUNK])
            vtiles.append(vt)

        for l in range(L):
            w1 = w116[:, l * H:(l + 1) * H]
            w2 = w2sb[:, l * H:(l + 1) * H]
            w3a = w3ab[:, l * 4:(l + 1) * 4]
            w3b = w3ab[:, 32 + l * 4:32 + (l + 1) * 4]
            mw = mw16[:, l * 4:(l + 1) * 4]
            b1ap = bbf[:, OB1 + l:OB1 + l + 1]
            b2ap = bbf[:, OB2 + l:OB2 + l + 1]
            btap = btf[:, l:l + 1]
            cfap = cff[:, l:l + 1]

            for half in range(2):
                tB = batp.tile([128, HALF * CHUNK], f32, tag="tB")
                a2B = batp.tile([128, HALF * CHUNK], f16, tag="a2B")
                tiles = range(half * HALF, (half + 1) * HALF)
                # ---- pass 1: gelu/tanh table set ----
                for t in tiles:
                    toff = (t - half * HALF) * CHUNK
                    xsp = vtiles[t]
                    h1 = hp.tile([128, TILE], f16, tag="h1")
                    for hh in range(2):
                        pre = pre_pool.tile([128, 1024], f32, tag="pre")
                        for jj in range(2):
                            j = hh * 2 + jj
                            nc.tensor.matmul(
                                pre[:, jj * 512:(jj + 1) * 512],
                                w1[32 * j:32 * j + 2, :],
                                xsp[32 * j:32 * j + 2, :],
                                start=True, stop=True,
                                tile_position=(32 * j, 0))
                        nc.scalar.activation(
                            h1[:, hh * 1024:(hh + 1) * 1024], pre[:, :],
                            AF.Gelu, bias=b1ap, scale=1.0)
                    h2 = hp.tile([128, TILE], f16, tag="h2")
                    for hh in range(2):
                        pre = pre_pool.tile([128, 1024], f32, tag="pre")
                        for jj in range(2):
                            j = hh * 2 + jj
                            nc.tensor.matmul(
                                pre[:, jj * 512:(jj + 1) * 512],
                                w2,
                                h1[:, j * 512:(j + 1) * 512],
                                start=True, stop=True)
                        nc.scalar.activation(
                            h2[:, hh * 1024:(hh + 1) * 1024], pre[:, :],
                            AF.Gelu, bias=b2ap, scale=1.0)
                    a1ps = sm_pool.tile([128, CHUNK], f32, tag="a1")
                    a2ps = sm_pool.tile([128, CHUNK], f32, tag="a2")
                    for j in range(4):
                        nc.tensor.matmul(
                            a1ps[32 * j:32 * j + 4, :], w3a,
                            h2[:, j * 512:(j + 1) * 512],
                            start=True, stop=True, tile_position=(0, 32 * j))
                    for j in range(4):
                        nc.tensor.matmul(
                            a2ps[32 * j:32 * j + 4, :], w3b,
                            h2[:, j * 512:(j + 1) * 512],
                            start=True, stop=True, tile_position=(0, 32 * j))
                    nc.scalar.activation(tB[:, toff:toff + CHUNK], a1ps[:, :],
                                         AF.Tanh, bias=btap, scale=0.1)
                    nc.vector.tensor_copy(a2B[:, toff:toff + CHUNK], a2ps[:, :])
                # ---- pass 2: exp table set ----
                for t in tiles:
                    toff = (t - half * HALF) * CHUNK
                    vt = vtiles[t]
                    esp = scr.tile([128, CHUNK], f16, tag="esp")
                    nc.scalar.activation(esp[:, :], tB[:, toff:toff + CHUNK],
                                         AF.Exp, scale=2.0)
                    xe = scr.tile([128, CHUNK], f16, tag="xe")
                    nc.vector.tensor_mul(xe[:, :], vt[:, :], esp[:, :])
                    # x1 rows: e==1 and a2==0, so this leaves x1 intact
                    nc.vector.tensor_add(vt[:, :], xe[:, :],
                                         a2B[:, toff:toff + CHUNK])
                    vops = out_pool.tile([128, CHUNK], f32, tag="vo")
                    for j in range(4):
                        nc.tensor.matmul(
                            vops[32 * j:32 * j + 4, :],
                            mw[32 * j:32 * j + 4, :],
                            vt[32 * j:32 * j + 4, :],
                            start=True, stop=True,
                            tile_position=(32 * j, 32 * j))
                    nc.vector.tensor_scalar_add(vt[:, :], vops[:, :], cfap)
                    if l == L - 1:
                        for j in range(NCHUNK):
                            c0 = t * TILE + j * CHUNK
                            nc.sync.dma_start(out=z3[:, c0:c0 + CHUNK],
                                              in_=vt[32 * j:32 * j + 3, :])
    return nc


def _get_program():
    global _PROGRAM
    if _PROGRAM is None:
        nc = _build_program()
        fixed, _ = _strip_pe_self_waits(nc.to_json_bytes())
        nc.to_json_bytes = lambda: fixed
        _PROGRAM = nc
    return _PROGRAM


LAST_EXEC_NS = None


def kernel(XYZ, W1, b1, W2, b2, W3, b3, g, off, P):
    global LAST_EXEC_NS
    _set_jax_cache()
    from concourse import bass_utils

    XYZ = np.asarray(XYZ, np.float32)
    w28, wrow = _pack_weights(np.asarray(W1), np.asarray(b1), np.asarray(W2),
                              np.asarray(b2), np.asarray(W3), np.asarray(b3),
                              np.asarray(g), np.asarray(off), np.asarray(P))
    XT = XYZ.T.astype(np.float16)        # [3, B], contiguous
    in_maps = [{"xt": XT[:, c * NC:(c + 1) * NC], "w28": w28, "wrow": wrow}
               for c in range(NCORES)]

    nc = _get_program()
    trace = bool(int(os.environ.get("COLORINN_TRACE", "0")))
    res = bass_utils.run_bass_kernel_spmd(
        nc, in_maps, core_ids=list(range(NCORES)), trace=trace)
    LAST_EXEC_NS = res.exec_time_ns

    out = np.empty((B, 3), np.float32)
    for c in range(NCORES):
        out[c * NC:(c + 1) * NC] = res.results[c]["z3"].T
    return out


# revision 9
# speedup vs baseline: 14.7536x; 1.0175x over previous
"""ColorINN forward kernel for 8 Trainium2 NeuronCores (pure data parallel).

Strategy:
- Batch B=524288 split evenly over 8 cores (Nc=65536 each), SPMD.
- Per core, the 4-feature coupling state stays SBUF-resident all 8 blocks
  as 32 per-tile [128, 512] fp16 tiles in a "span layout": partition
  32*j + r holds feature r of chunk j (chunk = 512 samples), so all small
  elementwise coupling work runs as full-width [128, 512] tiles and the
  only DRAM traffic is the initial load and final store.
- Each of the 8 coupling blocks runs as two passes over all tiles so the ACT
  table set only swaps twice per block (gelu+tanh set, then exp set):
    pass 1: L1 (K=2, row-packed via tile_position) -> gelu -> W2 (128x128)
            -> gelu -> W3a/W3b (M=4, col-strip packed) -> tanh -> stash
    pass 2: exp -> coupling mul/add -> 4x4 permute matmul (diagonal packed)
            -> +c bias -> store next state
- Host <-> device traffic is minimized (it rides a slow tunnel; the wall
  clock is ~97% host/transfer/RPC, the device kernel itself is ~6ms):
  inputs ship as XYZ^T in fp16 [3, Nc], W2 as fp8-e4m3 (upcast to fp16
  on-device), and everything else in one compact fp16 stack; the span
  layout and the strip-replicated weight tiles are built on-device with
  small DMAs. Output returns as [3, Nc] fp16.
- The JAX persistent compilation cache is enabled so repeat calls skip the
  per-call XLA executable rebuild (the jit closure is fresh each call).
- Matmuls run in fp16 (W2 quantized to fp8 for transfer only). Measured on
  hardware: rel err ~6.6e-3 on an output scale of ~7.8 (gate 2e-2). A
  post-trace BIR pass legalizes sync waits for walrus codegen's
  one-wait-per-instruction caps (PE-self waits on matmuls are dropped as
  redundant; other overflow waits move to injected single-wait
  EventSemaphore instructions on the same engine).
"""

import os
import numpy as np

L = 8
H = 128
B = 524288
NCORES = 8
NC = B // NCORES          # samples per core
CHUNK = 512               # samples per chunk (one matmul stream / psum bank)
NCHUNK = 4                # chunks packed across partition strips
TILE = CHUNK * NCHUNK     # 2048 samples per tile
NT = NC // TILE           # 32 tiles per pass
HALF = NT // 2            # tiles per half-pass (bounds SBUF batch size)

# w28 (fp8 e4m3): [128, L*H] W2 lhsT per block (upcast to fp16 on-device)
# wrow (fp16) column layout
OB1 = 0            # 8 cols: b1 per block (dense 128 rows; cast f32 on-device)
OB2 = 8            # 8 cols: b2
OW3 = 16           # 32 cols: l*4 + {W3[l][0], W3[l][1], .1*W3[l][2], .1*W3[l][3]}
OMW = 48           # 32 cols, rows 0-3: M_mat per block (strip-expanded on-device)
OBT = 80           # 8 cols, rows 0-1: 0.1*b3[l][0:2] (-> strip rows +2,+3)
OCF = 88           # 8 cols, rows 0-3: folded output bias
OW1 = 96           # 128 cols, rows 0-15: row 2l+r = W1[l].T[r] (strip-expanded)
WRCOLS = 224


def _softplus(x, beta=1.0):
    x = np.asarray(x, np.float64)
    return np.log1p(np.exp(-np.abs(beta * x))) / beta + np.maximum(x, 0.0)


def _pack_weights(W1, b1, W2, b2, W3, b3, g, off, P):
    """Host-side constant folding -> compact fp8 + fp16 stacks."""
    import ml_dtypes
    w28 = np.zeros((128, L * H), ml_dtypes.float8_e4m3)
    wrow = np.zeros((128, WRCOLS), np.float16)
    for l in range(L):
        scale = 0.2 * _softplus(0.5 * g[l].astype(np.float64))          # (4,)
        M_mat = scale[:, None] * P[l].astype(np.float64).T              # [i,m] = scale_i * P[m,i]
        c = off[l].astype(np.float64) @ P[l].astype(np.float64).T
        b3s = 0.1 * b3[l].astype(np.float64)
        c_fold = c + np.array([0, 0, b3s[2], b3s[3]]) @ M_mat
        w28[:, l * H:(l + 1) * H] = W2[l].T.astype(ml_dtypes.float8_e4m3)
        wrow[2 * l:2 * l + 2, OW1:] = W1[l].T
        wrow[:, OB1 + l] = b1[l]
        wrow[:, OB2 + l] = b2[l]
        wrow[:, OW3 + l * 4 + 0] = W3[l][0]
        wrow[:, OW3 + l * 4 + 1] = W3[l][1]
        wrow[:, OW3 + l * 4 + 2] = 0.1 * W3[l][2]
        wrow[:, OW3 + l * 4 + 3] = 0.1 * W3[l][3]
        wrow[0:4, OMW + l * 4:OMW + (l + 1) * 4] = M_mat.astype(np.float16)
        wrow[0:2, OBT + l] = (0.1 * b3[l][0:2]).astype(np.float16)
        wrow[0:4, OCF + l] = c_fold.astype(np.float16)
    return w28, wrow


_PROGRAM = None
_JAX_CACHE_SET = False


def _set_jax_cache():
    """Persistent XLA compilation cache: repeat kernel() calls rebuild the
    jit closure inside run_bass_kernel_spmd, so without this every call
    pays ~0.7s of executable rebuild."""
    global _JAX_CACHE_SET
    if _JAX_CACHE_SET:
        return
    try:
        import jax
        jax.config.update("jax_compilation_cache_dir", "/tmp/colorinn_jaxcache")
        jax.config.update("jax_persistent_cache_min_compile_time_secs", 0.0)
        jax.config.update("jax_persistent_cache_min_entry_size_bytes", -1)
    except Exception:
        pass
    _JAX_CACHE_SET = True


def _strip_pe_self_waits(bj_bytes):
    """Legalize sync waits for walrus codegen wait-slot caps.

    Most TRN2 instruction structs accept only one attached sync wait
    (Activation takes two). Tile can emit more. Two fixes, applied in order:
    - Matmults drop PE-self waits (PSUM WAW between matmuls is already
      guaranteed by in-order matmul completion on TRN2).
    - Any remaining overflow waits move onto an injected same-engine
      EventSemaphore placed immediately before the instruction.
    """
    import json
    bj = json.loads(bj_bytes)
    caps = {"EventSemaphore": 99, "Call": 99}
    nes = 0
    for f in bj["functions"]:
        for blk in f["blocks"]:
            out_insts = []
            for ins in blk["instructions"]:
                si = ins.get("sync_info") or {}
                w = si.get("on_wait") or []
                op = ins.get("opcode")
                if op == "Matmult" and len(w) >= 2:
                    w = [x for x in w
                         if not x.get("ant_name", "").startswith("PE")]
                    si["on_wait"] = w
                cap = caps.get(op, 1)
                if len(w) > cap:
                    keep = w[-cap:] if cap else []
                    moved = w[:-cap] if cap else list(w)
                    si["on_wait"] = keep
                    for mv in moved:
                        nes += 1
                        out_insts.append({
                            "debug": ins.get("debug", 0),
                            "engine": ins.get("engine"),
                            "ins": [], "outs": [],
                            "name": f"eswait_{nes}",
                            "opcode": "EventSemaphore",
                            "sync_info": {"on_update": [], "on_wait": [mv]},
                        })
                out_insts.append(ins)
            blk["instructions"] = out_insts
    return json.dumps(bj).encode(), nes


def _build_program():
    import concourse.bass as bass
    import concourse.tile as tile
    import concourse.mybir as mybir
    from contextlib import ExitStack

    f32 = mybir.dt.float32
    f16 = mybir.dt.float16
    f8 = mybir.dt.float8e4
    AF = mybir.ActivationFunctionType

    nc = bass.Bass("TRN2", target_bir_lowering=False, debug=False)
    xt = nc.dram_tensor("xt", [3, NC], f16, kind="ExternalInput").ap()
    w2d = nc.dram_tensor("w28", [128, L * H], f8, kind="ExternalInput").ap()
    wrd = nc.dram_tensor("wrow", [128, WRCOLS], f16, kind="ExternalInput").ap()
    z3 = nc.dram_tensor("z3", [3, NC], f16, kind="ExternalOutput").ap()

    with tile.TileContext(nc) as tc, ExitStack() as ctx:
        consts = ctx.enter_context(tc.tile_pool(name="consts", bufs=1))
        scr = ctx.enter_context(tc.tile_pool(name="scr", bufs=3))
        vtp = ctx.enter_context(tc.tile_pool(name="vt", bufs=1))
        hp = ctx.enter_context(tc.tile_pool(name="hp", bufs=2))
        batp = ctx.enter_context(tc.tile_pool(name="bat", bufs=1))
        pre_pool = ctx.enter_context(tc.tile_pool(name="pre", bufs=2, space="PSUM"))
        sm_pool = ctx.enter_context(tc.tile_pool(name="sm", bufs=1, space="PSUM"))
        out_pool = ctx.enter_context(tc.tile_pool(name="po", bufs=2, space="PSUM"))

        # ---- weight load + on-device expansion ----
        w28sb = consts.tile([128, L * H], f8)
        nc.sync.dma_start(out=w28sb[:, :], in_=w2d[:, :])
        wrsb = consts.tile([128, WRCOLS], f16)
        nc.sync.dma_start(out=wrsb[:, :], in_=wrd[:, :])

        # upcast W2 fp8 -> fp16 for the matmuls
        w2sb = consts.tile([128, L * H], f16)
        nc.vector.tensor_copy(w2sb[:, :], w28sb[:, :])

        # tiny ops consuming the weight DMAs so their waits land here once,
        # not on the first real instruction of every engine epoch
        warm = pre_pool.tile([128, 1024], f32, tag="pre")
        nc.tensor.matmul(warm[0:2, 0:2], w2sb[0:2, 0:2], w2sb[0:2, 0:2],
                         start=True, stop=True)
        warmsb = consts.tile([128, 2], f32)
        nc.scalar.copy(warmsb[0:1, 0:1], wrsb[0:1, 0:1])

        # biases to f32 for the ACT bias APs
        bbf = consts.tile([128, 16], f32)
        nc.vector.tensor_copy(bbf[:, :], wrsb[:, OB1:OB1 + 16])
        # bt/cf compact rows cast to f32 (strip-expanded below)
        btcf = consts.tile([128, 16], f32)
        nc.vector.tensor_copy(btcf[0:2, 0:L], wrsb[0:2, OBT:OBT + L])
        nc.vector.tensor_copy(btcf[0:4, 8:8 + L], wrsb[0:4, OCF:OCF + L])

        # W1 lhsT rows {32j, 32j+1} per block, from compact rows 2l+r
        w116 = consts.tile([128, L * H], f16)
        for l in range(L):
            for j in range(NCHUNK):
                nc.scalar.dma_start(
                    out=w116[32 * j:32 * j + 2, l * H:(l + 1) * H],
                    in_=wrsb[2 * l:2 * l + 2, OW1:])
        # W3a/W3b lhsT [128, 4] per block: cols 0,1 zero; col 2+r = W3-row
        # (a outputs land on rows {32j+2, 32j+3}, aligned with x2 in the span)
        w3ab = consts.tile([128, 64], f16)
        nc.vector.memset(w3ab[:, :], 0.0)
        for l in range(L):
            nc.vector.tensor_copy(w3ab[:, l * 4 + 2:l * 4 + 4],
                                  wrsb[:, OW3 + l * 4:OW3 + l * 4 + 2])
            nc.vector.tensor_copy(w3ab[:, 32 + l * 4 + 2:32 + l * 4 + 4],
                                  wrsb[:, OW3 + l * 4 + 2:OW3 + l * 4 + 4])
        # P-matmul lhsT rows {32j..32j+3}: M_mat, strip-replicated
        mw16 = consts.tile([128, 32], f16)
        nc.vector.tensor_copy(mw16[0:4, :], wrsb[0:4, OMW:OMW + 32])
        for j in range(1, NCHUNK):
            nc.sync.dma_start(out=mw16[32 * j:32 * j + 4, :], in_=mw16[0:4, :])
        # tanh bias rows {32j+2, 32j+3} = 0.1*b3[0:2]; elsewhere 0 so the
        # x1 rows see tanh(0)=0 -> exp=1 (x1 passthrough trick)
        btf = consts.tile([128, L], f32)
        nc.vector.memset(btf[:, :], 0.0)
        cff = consts.tile([128, L], f32)
        nc.vector.memset(cff[:, :], 0.0)
        for j in range(NCHUNK):
            nc.sync.dma_start(out=btf[32 * j + 2:32 * j + 4, :],
                              in_=btcf[0:2, 0:L])
            nc.sync.dma_start(out=cff[32 * j:32 * j + 4, :],
                              in_=btcf[0:4, 8:8 + L])

        # ---- input load: span layout built by DMA from XYZ^T fp16 ----
        vtiles = []
        for t in range(NT):
            vt = vtp.tile([128, CHUNK], f16, tag=f"v{t}")
            nc.vector.memset(vt[:, :], 0.0)   # pad feature row starts at 0
            for j in range(NCHUNK):
                c0 = t * TILE + j * CHUNK
                nc.sync.dma_start(out=vt[32 * j:32 * j + 3, :],
                                  in_=xt[:, c0:c0 + CHUNK])
            vtiles.append(vt)

        for l in range(L):
            w1 = w116[:, l * H:(l + 1) * H]
            w2 = w2sb[:, l * H:(l + 1) * H]
            w3a = w3ab[:, l * 4:(l + 1) * 4]
            w3b = w3ab[:, 32 + l * 4:32 + (l + 1) * 4]
            mw = mw16[:, l * 4:(l + 1) * 4]
            b1ap = bbf[:, OB1 + l:OB1 + l + 1]
            b2ap = bbf[:, OB2 + l:OB2 + l + 1]
            btap = btf[:, l:l + 1]
            cfap = cff[:, l:l + 1]

            for half in range(2):
                tB = batp.tile([128, HALF * CHUNK], f32, tag="tB")
                a2B = batp.tile([128, HALF * CHUNK], f16, tag="a2B")
                tiles = range(half * HALF, (half + 1) * HALF)
                # ---- pass 1: gelu/tanh table set ----
                for t in tiles:
                    toff = (t - half * HALF) * CHUNK
                    xsp = vtiles[t]
                    h1 = hp.tile([128, TILE], f16, tag="h1")
                    for hh in range(2):
                        pre = pre_pool.tile([128, 1024], f32, tag="pre")
                        for jj in range(2):
                            j = hh * 2 + jj
                            nc.tensor.matmul(
                                pre[:, jj * 512:(jj + 1) * 512],
                                w1[32 * j:32 * j + 2, :],
                                xsp[32 * j:32 * j + 2, :],
                                start=True, stop=True,
                                tile_position=(32 * j, 0))
                        nc.scalar.activation(
                            h1[:, hh * 1024:(hh + 1) * 1024], pre[:, :],
                            AF.Gelu, bias=b1ap, scale=1.0)
                    h2 = hp.tile([128, TILE], f16, tag="h2")
                    for hh in range(2):
                        pre = pre_pool.tile([128, 1024], f32, tag="pre")
                        for jj in range(2):
                            j = hh * 2 + jj
                            nc.tensor.matmul(
                                pre[:, jj * 512:(jj + 1) * 512],
                                w2,
                                h1[:, j * 512:(j + 1) * 512],
                                start=True, stop=True)
                        nc.scalar.activation(
                            h2[:, hh * 1024:(hh + 1) * 1024], pre[:, :],
                            AF.Gelu, bias=b2ap, scale=1.0)
                    a1ps = sm_pool.tile([128, CHUNK], f32, tag="a1")
                    a2ps = sm_pool.tile([128, CHUNK], f32, tag="a2")
                    for j in range(4):
                        nc.tensor.matmul(
                            a1ps[32 * j:32 * j + 4, :], w3a,
                            h2[:, j * 512:(j + 1) * 512],
                            start=True, stop=True, tile_position=(0, 32 * j))
                    for j in range(4):
                        nc.tensor.matmul(
                            a2ps[32 * j:32 * j + 4, :], w3b,
                            h2[:, j * 512:(j + 1) * 512],
                            start=True, stop=True, tile_position=(0, 32 * j))
                    nc.scalar.activation(tB[:, toff:toff + CHUNK], a1ps[:, :],
                                         AF.Tanh, bias=btap, scale=0.1)
                    nc.vector.tensor_copy(a2B[:, toff:toff + CHUNK], a2ps[:, :])
                # ---- pass 2: exp table set ----
                for t in tiles:
                    toff = (t - half * HALF) * CHUNK
                    vt = vtiles[t]
                    esp = scr.tile([128, CHUNK], f16, tag="esp")
                    nc.scalar.activation(esp[:, :], tB[:, toff:toff + CHUNK],
                                         AF.Exp, scale=2.0)
                    xe = scr.tile([128, CHUNK], f16, tag="xe")
                    nc.vector.tensor_mul(xe[:, :], vt[:, :], esp[:, :])
                    # x1 rows: e==1 and a2==0, so this leaves x1 intact
                    nc.vector.tensor_add(vt[:, :], xe[:, :],
                                         a2B[:, toff:toff + CHUNK])
                    vops = out_pool.tile([128, CHUNK], f32, tag="vo")
                    for j in range(4):
                        nc.tensor.matmul(
                            vops[32 * j:32 * j + 4, :],
                            mw[32 * j:32 * j + 4, :],
                            vt[32 * j:32 * j + 4, :],
                            start=True, stop=True,
                            tile_position=(32 * j, 32 * j))
                    nc.vector.tensor_scalar_add(vt[:, :], vops[:, :], cfap)
                    if l == L - 1:
                        for j in range(NCHUNK):
                            c0 = t * TILE + j * CHUNK
                            nc.sync.dma_start(out=z3[:, c0:c0 + CHUNK],
                                              in_=vt[32 * j:32 * j + 3, :])
    return nc


def _get_program():
    global _PROGRAM
    if _PROGRAM is None:
        nc = _build_program()
        fixed, _ = _strip_pe_self_waits(nc.to_json_bytes())
        nc.to_json_bytes = lambda: fixed
        _PROGRAM = nc
    return _PROGRAM


LAST_EXEC_NS = None


def kernel(XYZ, W1, b1, W2, b2, W3, b3, g, off, P):
    global LAST_EXEC_NS
    _set_jax_cache()
    from concourse import bass_utils

    XYZ = np.asarray(XYZ, np.float32)
    w28, wrow = _pack_weights(np.asarray(W1), np.asarray(b1), np.asarray(W2),
                              np.asarray(b2), np.asarray(W3), np.asarray(b3),
                              np.asarray(g), np.asarray(off), np.asarray(P))
    XT = XYZ.T.astype(np.float16)        # [3, B], contiguous
    in_maps = [{"xt": XT[:, c * NC:(c + 1) * NC], "w28": w28, "wrow": wrow}
               for c in range(NCORES)]

    nc = _get_program()
    trace = bool(int(os.environ.get("COLORINN_TRACE", "0")))
    res = bass_utils.run_bass_kernel_spmd(
        nc, in_maps, core_ids=list(range(NCORES)), trace=trace)
    LAST_EXEC_NS = res.exec_time_ns

    out = np.empty((B, 3), np.float32)
    for c in range(NCORES):
        out[c * NC:(c + 1) * NC] = res.results[c]["z3"].T
    return out
